# revision 8
# baseline (speedup 1.0000x reference)
"""DeepseekV2 MoE kernel for 8 trn2 NeuronCores (expert-parallel).

Strategy:
  - Router (gate matmul + softmax + group-limited top-k) runs on host in
    jax-on-CPU, replicating the module's math op-for-op.
  - Tokens are gathered per expert and dispatched expert-parallel: each
    core owns 4 expert slots. Experts are assigned to slots by sorted
    token count (rank-balanced across cores), with per-slot compile-time
    capacities CAPS=[224,208,192,184] covering the actual per-rank loads
    (~214 max for T=1024/K=6/E=32).
  - Each core runs silu(x@w1.T)*(x@w3.T)@w2.T for its 4 slots, tokens on
    the matmul free dim so no on-device transposes are needed.
  - Weight DMAs are coalesced to ~1-1.4MB per dma_start, all in program
    order on the sync HWDGE ring (the kernel is bound by the ~330GB/s
    per-core HBM read bandwidth; splitting weights across rings only
    adds latency). x loads ride the scalar ring and y stores SWDGE so
    they overlap the weight stream without reordering it.
  - Host scatter-adds the per-(token,expert) outputs with the routing
    weights. Capacity overflow (never hit for the target shapes) is
    computed on host as a correctness fallback.
"""

import os
import numpy as np

import concourse.bass as bass
import concourse.mybir as mybir
import concourse.tile as tile
from concourse import bacc

E, G, TG, TOPK = 32, 8, 3, 6
H, I, T = 2048, 1408, 1024
N_CORES = 8
EPC = E // N_CORES          # expert slots per core
CAPS = [224, 208, 192, 184]  # per-slot token capacity (rank-balanced)
SOFF = [0]
for _c in CAPS[:-1]:
    SOFF.append(SOFF[-1] + _c)
TCAP = sum(CAPS)            # 816
KT, IT, HT = H // 128, I // 128, H // 128   # 16, 11, 16 k/i/h tiles
W2C = 4                     # w2 strips per DMA chunk
NW2C = HT // W2C            # 4 chunks

# matmul dtype mode: "f32" (rel err ~9e-7), "f32r" (~2.6e-4), "f16"
# (~5.1e-4), "bf16". f16 fastest at acceptable precision here.
MM_MODE = os.environ.get("MOE_MM_MODE", "f16")

_prog_cache = {}


def _build_program(mode, repeat=1, loop_reps=0):
    """Per-core SPMD program: 4 expert slots x (CAPS[s] tokens) gated FFN.

    repeat>1 re-runs the whole computation unrolled (identical outputs);
    loop_reps>0 wraps it in a hardware For_i loop instead. Both exist so
    wall-time deltas isolate device time from dispatch overhead when
    profiling."""
    f32 = mybir.dt.float32
    store_dt = {"bf16": mybir.dt.bfloat16,
                "f16": mybir.dt.float16,
                "f32r": mybir.dt.float32r}.get(mode, f32)

    nc = bacc.Bacc("TRN2", target_bir_lowering=False, debug=False,
                   num_devices=N_CORES)

    # Blocked layouts (see host prep below):
    #   xb   [128, KT*TCAP]   slot s block at col SOFF[s]*KT, inside it
    #                         [k][t]: xb[p, SOFF[s]*KT + k*CAPS[s] + t]
    #                           = x_slot_s[t, 128k+p]
    #   w13b [EPC, IT, 128, 2*KT*128]
    #        j < 2048: w1[slot, 128it+(j%128), 128*(j//128)+p]
    #        j >= 2048: same for w3 (j-2048)
    #   w2b  [EPC, NW2C, 128, W2C*IT*128]
    #        jj = hl*1408 + it*128 + hsub:
    #          w2[slot, 128*(W2C*c+hl)+hsub, 128it+p]
    #   yb   [128, HT*TCAP]   slot s block at col HT*SOFF[s]:
    #        yb[p, HT*SOFF[s] + ht*CAPS[s] + t] = y_slot_s[t, 128ht+p]
    xb = nc.dram_tensor("xb", [128, KT * TCAP], store_dt,
                        kind="ExternalInput").ap()
    w13b = nc.dram_tensor("w13b", [EPC, IT, 128, 2 * KT * 128], store_dt,
                          kind="ExternalInput").ap()
    w2b = nc.dram_tensor("w2b", [EPC, NW2C, 128, W2C * IT * 128], store_dt,
                         kind="ExternalInput").ap()
    yb = nc.dram_tensor("yb", [128, HT * TCAP], store_dt,
                        kind="ExternalOutput").ap()

    with tile.TileContext(nc) as tc:
        with (
            tc.tile_pool(name="xpool", bufs=1) as xpool,
            tc.tile_pool(name="wpool", bufs=6) as wpool,
            tc.tile_pool(name="w2pool", bufs=3) as w2pool,
            tc.tile_pool(name="hhpool", bufs=2 * IT + 1) as hhpool,
            tc.tile_pool(name="evpool", bufs=4) as evpool,
            tc.tile_pool(name="ypool", bufs=2) as ypool,
            tc.tile_pool(name="psAB", bufs=6, space="PSUM") as psAB,
            tc.tile_pool(name="psY", bufs=2, space="PSUM") as psYp,
        ):
            # Resident gathered activations, one tile per slot so matmuls
            # only wait on their own slot's load.
            x_sb = [xpool.tile([128, KT * CAPS[s]], store_dt, tag=f"x{s}",
                               name=f"x_sb{s}")
                    for s in range(EPC)]
            for s in range(EPC):
                nc.scalar.dma_start(
                    x_sb[s][:], xb[:, bass.ds(SOFF[s] * KT, KT * CAPS[s])])

            def body():
              for e in range(EPC):
                cap = CAPS[e]
                hh = []
                for it in range(IT):
                    w13s = wpool.tile([128, 2 * KT * 128], store_dt,
                                      tag="w13s")
                    nc.sync.dma_start(w13s[:], w13b[e, it])

                    psA = psAB.tile([128, cap], f32, tag="ps")
                    for k in range(KT):
                        nc.tensor.matmul(
                            psA[:],
                            w13s[:, bass.ts(k, 128)],
                            x_sb[e][:, bass.ds(k * cap, cap)],
                            start=(k == 0), stop=(k == KT - 1))
                    psB = psAB.tile([128, cap], f32, tag="ps")
                    for k in range(KT):
                        nc.tensor.matmul(
                            psB[:],
                            w13s[:, bass.ds(KT * 128 + k * 128, 128)],
                            x_sb[e][:, bass.ds(k * cap, cap)],
                            start=(k == 0), stop=(k == KT - 1))

                    sA = evpool.tile([128, cap], f32, tag="silu")
                    nc.scalar.activation(
                        sA[:], psA[:], mybir.ActivationFunctionType.Silu)
                    hh_t = hhpool.tile([128, cap], store_dt, tag="hh")
                    nc.vector.tensor_mul(hh_t[:], sA[:], psB[:])
                    hh.append(hh_t)

                y_sb = ypool.tile([128, HT * cap], store_dt, tag="y")
                for c in range(NW2C):
                    w2s = w2pool.tile([128, W2C * IT * 128], store_dt,
                                      tag="w2s")
                    nc.sync.dma_start(w2s[:], w2b[e, c])
                    for hl in range(W2C):
                        psy = psYp.tile([128, cap], f32, tag="psy")
                        for it2 in range(IT):
                            nc.tensor.matmul(
                                psy[:],
                                w2s[:, bass.ds(hl * IT * 128 + it2 * 128,
                                               128)],
                                hh[it2][:],
                                start=(it2 == 0), stop=(it2 == IT - 1))
                        ht = c * W2C + hl
                        nc.vector.tensor_copy(
                            y_sb[:, bass.ds(ht * cap, cap)], psy[:])
                nc.gpsimd.dma_start(
                    yb[:, bass.ds(HT * SOFF[e], HT * cap)], y_sb[:])

            if loop_reps > 0:
                with tc.For_i(0, loop_reps, 1,
                              hint_engines=tuple(mybir.ALL_ENGINES),
                              staggered_reset=True):
                    body()
            else:
                for _ in range(repeat):
                    body()
    nc.compile()
    return nc


def get_program(mode=None, repeat=1, loop_reps=0):
    mode = mode or MM_MODE
    key = (mode, repeat, loop_reps)
    if key not in _prog_cache:
        _prog_cache[key] = _build_program(mode, repeat, loop_reps)
    return _prog_cache[key]


_exec_cache = {}


def get_executor(mode=None, repeat=1, loop_reps=0):
    """Build (once) a PJRT executable for the SPMD program. Returns a
    callable: in_maps (list of per-core dicts) -> list of per-core output
    dicts."""
    mode = mode or MM_MODE
    key = (mode, repeat, loop_reps)
    if key in _exec_cache:
        return _exec_cache[key]

    import jax
    from jax.sharding import Mesh, NamedSharding, PartitionSpec
    from jax.experimental.shard_map import shard_map
    from concourse import bass2jax

    bass2jax.install_neuronx_cc_hook()
    nc = get_program(mode, repeat, loop_reps)

    partition_name = (nc.partition_id_tensor.name
                      if nc.partition_id_tensor else None)
    in_names, out_names, out_avals, out_shapes = [], [], [], []
    for alloc in nc.m.functions[0].allocations:
        if not isinstance(alloc, mybir.MemoryLocationSet):
            continue
        name = alloc.memorylocations[0].name
        if alloc.kind == "ExternalInput":
            if name != partition_name:
                in_names.append(name)
        elif alloc.kind == "ExternalOutput":
            shape = tuple(alloc.tensor_shape)
            dtype = mybir.dt.np(alloc.dtype)
            out_names.append(name)
            out_avals.append(jax.core.ShapedArray(shape, dtype))
            out_shapes.append((shape, dtype))
    n_params = len(in_names)
    n_outs = len(out_avals)
    all_in_names = in_names + out_names + (
        [partition_name] if partition_name else [])

    def _body(*args):
        operands = list(args)
        if partition_name is not None:
            operands.append(bass2jax.partition_id_tensor())
        return tuple(bass2jax._bass_exec_p.bind(
            *operands,
            out_avals=tuple(out_avals),
            in_names=tuple(all_in_names),
            out_names=tuple(out_names),
            lowering_input_output_aliases=(),
            sim_require_finite=True,
            sim_require_nnan=True,
            nc=nc,
        ))

    devices = jax.devices()[:N_CORES]
    mesh = Mesh(np.asarray(devices), ("core",))
    sharded = jax.jit(
        shard_map(_body, mesh=mesh,
                  in_specs=(PartitionSpec("core"),) * (n_params + n_outs),
                  out_specs=(PartitionSpec("core"),) * n_outs,
                  check_rep=False),
        donate_argnums=tuple(range(n_params, n_params + n_outs)),
        keep_unused=True)
    shard = NamedSharding(mesh, PartitionSpec("core"))

    def run(in_maps):
        concat_in = [
            np.concatenate([np.asarray(in_maps[c][nm])
                            for c in range(N_CORES)], axis=0)
            for nm in in_names]
        zeros = [np.zeros((N_CORES * s[0], *s[1:]), d)
                 for (s, d) in out_shapes]
        outs = sharded(*[jax.device_put(a, shard) for a in concat_in],
                       *[jax.device_put(z, shard) for z in zeros])
        return [
            {name: np.asarray(outs[i]).reshape(N_CORES, *out_avals[i].shape)[c]
             for i, name in enumerate(out_names)}
            for c in range(N_CORES)]

    run.in_names = in_names
    run.out_names = out_names
    run.out_shapes = out_shapes
    run.sharded = sharded
    run.shard = shard
    _exec_cache[key] = run
    return run


def _route(hidden_states, gate_weight):
    """Replicates the module's router on CPU via jax (bit-compatible with
    the reference implementation)."""
    import jax
    import jax.numpy as jnp
    cpu = jax.devices("cpu")[0]
    with jax.default_device(cpu):
        hs = jnp.asarray(hidden_states)
        gw = jnp.asarray(gate_weight)
        logits = hs @ gw.T
        probs = jax.nn.softmax(logits.astype(jnp.float32), axis=-1)
        group_scores = probs.reshape(T, G, E // G).max(axis=-1)
        _, gidx = jax.lax.top_k(group_scores, TG)
        rows = jnp.arange(T)[:, None]
        gmask = jnp.zeros((T, G), probs.dtype).at[rows, gidx].set(1.0)
        smask = jnp.repeat(gmask, E // G, axis=1)
        tmp_scores = jnp.where(smask > 0, probs, 0.0)
        rw, sel = jax.lax.top_k(tmp_scores, TOPK)
        return np.asarray(sel), np.asarray(rw, dtype=np.float32)


def _np_store_dtype(mode):
    if mode == "bf16":
        import ml_dtypes
        return np.dtype(ml_dtypes.bfloat16)
    if mode == "f16":
        return np.dtype(np.float16)
    return np.dtype(np.float32)


def prep_inputs(hidden_states, w1_weight, w3_weight, w2_weight, sel, mode):
    """Balance experts to (core, slot) by sorted token count, gather
    tokens, and block weights for the device layout.
    Returns (in_maps, assign, overflow) where assign[t,k] = row in the
    global gathered array (core*TCAP + SOFF[slot] + pos) or -1."""
    sdt = _np_store_dtype(mode)
    counts = np.bincount(sel.reshape(-1), minlength=E)
    order = np.argsort(-counts, kind="stable")   # expert ids, desc count
    # expert_of[c][s] = expert id owned by core c, slot s
    expert_of = np.empty((N_CORES, EPC), dtype=np.int64)
    slot_of = np.empty(E, dtype=np.int64)        # expert -> flat slot id
    for s in range(EPC):
        for c in range(N_CORES):
            eid = order[s * N_CORES + c]
            expert_of[c, s] = eid
            slot_of[eid] = c * EPC + s

    assign = np.full((T, TOPK), -1, dtype=np.int64)
    fill = np.zeros(E, dtype=np.int64)
    overflow = []
    tok_of = [np.zeros(CAPS[s % EPC], dtype=np.int64)
              for s in range(N_CORES * EPC)]
    used = [np.zeros(CAPS[s % EPC], dtype=bool)
            for s in range(N_CORES * EPC)]
    for t in range(T):
        for k in range(TOPK):
            e = sel[t, k]
            fs = slot_of[e]
            c, s = fs // EPC, fs % EPC
            p = fill[e]
            if p < CAPS[s]:
                tok_of[fs][p] = t
                used[fs][p] = True
                fill[e] = p + 1
                assign[t, k] = c * TCAP + SOFF[s] + p
            else:
                overflow.append((t, k, e))

    in_maps = []
    for core in range(N_CORES):
        # gathered x per slot -> xb [128, KT*TCAP]
        xbc = np.zeros((128, KT * TCAP), dtype=np.float32)
        for s in range(EPC):
            fs = core * EPC + s
            xg = np.zeros((CAPS[s], H), dtype=np.float32)
            xg[used[fs]] = hidden_states[tok_of[fs][used[fs]]]
            # [t, H] -> [k, p, t] -> [p, k*cap+t]
            blk = (xg.T.reshape(KT, 128, CAPS[s])
                   .transpose(1, 0, 2).reshape(128, KT * CAPS[s]))
            xbc[:, SOFF[s] * KT:SOFF[s] * KT + KT * CAPS[s]] = blk
        xbc = np.ascontiguousarray(xbc).astype(sdt, copy=False)

        es = expert_of[core]                      # expert ids for slots
        # weights: w1/w3 [e, I, H] -> strips [s, it, p(h in kt), kt*128+i]
        w1c = (w1_weight[es].transpose(0, 2, 1)   # [s, H, I]
               .reshape(EPC, KT, 128, IT, 128)    # [s, kt, p, it, i]
               .transpose(0, 3, 2, 1, 4)          # [s, it, p, kt, i]
               .reshape(EPC, IT, 128, KT * 128))
        w3c = (w3_weight[es].transpose(0, 2, 1)
               .reshape(EPC, KT, 128, IT, 128)
               .transpose(0, 3, 2, 1, 4)
               .reshape(EPC, IT, 128, KT * 128))
        w13c = np.ascontiguousarray(
            np.concatenate([w1c, w3c], axis=-1)).astype(sdt, copy=False)
        # w2 [e, H, I] -> strips [s, ht, p(i in it), it*128+h] chunked by 4
        w2c = np.ascontiguousarray(
            w2_weight[es].transpose(0, 2, 1)      # [s, I, H]
            .reshape(EPC, IT, 128, HT, 128)       # [s, it, p, ht, h]
            .transpose(0, 3, 2, 1, 4)             # [s, ht, p, it, h]
            .reshape(EPC, NW2C, W2C, 128, IT * 128)   # [s, c, hl, p, ith]
            .transpose(0, 1, 3, 2, 4)             # [s, c, p, hl, ith]
            .reshape(EPC, NW2C, 128, W2C * IT * 128)).astype(sdt, copy=False)
        in_maps.append({"xb": xbc, "w13b": w13c, "w2b": w2c})
    return in_maps, assign, overflow


def combine(results, assign, rw, overflow, hidden_states,
            w1_weight, w3_weight, w2_weight):
    # Global gathered output rows: core-major [N_CORES*TCAP, H]
    ys = []
    for core in range(N_CORES):
        yc = results[core]["yb"].astype(np.float32)   # [128, HT*TCAP]
        ycore = np.empty((TCAP, H), dtype=np.float32)
        for s in range(EPC):
            blk = yc[:, HT * SOFF[s]:HT * SOFF[s] + HT * CAPS[s]]
            # [p, ht*cap+t] -> [t, ht, p] -> [t, H]
            ycore[SOFF[s]:SOFF[s] + CAPS[s]] = (
                blk.reshape(128, HT, CAPS[s]).transpose(2, 1, 0)
                .reshape(CAPS[s], H))
        ys.append(ycore)
    yg = np.concatenate(ys, axis=0)               # [N_CORES*TCAP, H]

    flat = assign.reshape(-1)
    ok = flat >= 0
    picked = np.zeros((T * TOPK, H), dtype=np.float32)
    picked[ok] = yg[flat[ok]]
    out = (picked.reshape(T, TOPK, H)
           * rw[:, :, None]).sum(axis=1).astype(np.float32)

    if overflow:
        for (t, k, e) in overflow:
            x = hidden_states[t]
            h = (x @ w1_weight[e].T)
            h = (h / (1.0 + np.exp(-h))) * (x @ w3_weight[e].T)
            out[t] += rw[t, k] * (h @ w2_weight[e].T)
    return out


def kernel(hidden_states, gate_weight, w1_weight, w3_weight, w2_weight):
    mode = MM_MODE
    runner = get_executor(mode)
    sel, rw = _route(hidden_states, gate_weight)
    in_maps, assign, overflow = prep_inputs(
        hidden_states, w1_weight, w3_weight, w2_weight, sel, mode)
    results = runner(in_maps)
    return combine(results, assign, rw, overflow, hidden_states,
                   w1_weight, w3_weight, w2_weight)


# revision 10
# speedup vs baseline: 1.0361x; 1.0361x over previous
"""DeepseekV2 MoE kernel for 8 trn2 NeuronCores (expert-parallel).

Strategy:
  - Router (gate matmul + softmax + group-limited top-k) runs on host in
    jax-on-CPU, replicating the module's math op-for-op.
  - Tokens are gathered per expert and dispatched expert-parallel: each
    core owns 4 expert slots. Experts are assigned to slots by sorted
    token count (rank-balanced across cores), with per-slot compile-time
    capacities CAPS=[224,208,192,184] covering the actual per-rank loads
    (~214 max for T=1024/K=6/E=32).
  - Each core runs silu(x@w1.T)*(x@w3.T)@w2.T for its 4 slots, tokens on
    the matmul free dim so no on-device transposes are needed.
  - Weight DMAs are coalesced to ~1-1.4MB per dma_start, all in program
    order on the sync HWDGE ring (the kernel is bound by the ~330GB/s
    per-core HBM read bandwidth; splitting weights across rings only
    adds latency). x loads ride the scalar ring and y stores SWDGE so
    they overlap the weight stream without reordering it.
  - Host scatter-adds the per-(token,expert) outputs with the routing
    weights. Capacity overflow (never hit for the target shapes) is
    computed on host as a correctness fallback.
"""

import os
import numpy as np

import concourse.bass as bass
import concourse.mybir as mybir
import concourse.tile as tile
from concourse import bacc

E, G, TG, TOPK = 32, 8, 3, 6
H, I, T = 2048, 1408, 1024
N_CORES = 8
EPC = E // N_CORES          # expert slots per core
CAPS = [224, 208, 192, 184]  # per-slot token capacity (rank-balanced)
SOFF = [0]
for _c in CAPS[:-1]:
    SOFF.append(SOFF[-1] + _c)
TCAP = sum(CAPS)            # 816
KT, IT, HT = H // 128, I // 128, H // 128   # 16, 11, 16 k/i/h tiles
W2C = 8                     # w2 strips per DMA chunk
NW2C = HT // W2C            # 2 chunks
W13CH = [(0, 1), (1, 2), (3, 2), (5, 2), (7, 2), (9, 2)]  # (it0, n) chunks

# matmul dtype mode: "f32" (rel err ~9e-7), "f32r" (~2.6e-4), "f16"
# (~5.1e-4), "bf16". f16 fastest at acceptable precision here.
MM_MODE = os.environ.get("MOE_MM_MODE", "f16")

_prog_cache = {}


def _build_program(mode, repeat=1, loop_reps=0):
    """Per-core SPMD program: 4 expert slots x (CAPS[s] tokens) gated FFN.

    repeat>1 re-runs the whole computation unrolled (identical outputs);
    loop_reps>0 wraps it in a hardware For_i loop instead. Both exist so
    wall-time deltas isolate device time from dispatch overhead when
    profiling."""
    f32 = mybir.dt.float32
    store_dt = {"bf16": mybir.dt.bfloat16,
                "f16": mybir.dt.float16,
                "f32r": mybir.dt.float32r}.get(mode, f32)

    nc = bacc.Bacc("TRN2", target_bir_lowering=False, debug=False,
                   num_devices=N_CORES)

    # Blocked layouts (see host prep below):
    #   xb   [128, KT*TCAP]   slot s block at col SOFF[s]*KT, inside it
    #                         [k][t]: xb[p, SOFF[s]*KT + k*CAPS[s] + t]
    #                           = x_slot_s[t, 128k+p]
    #   w13b [EPC, 128, IT*2*KT*128]  (partition-major; DMA'd in W13CH
    #        chunks of 1-2 its). col = it*4096 + j with
    #        j < 2048: w1[slot, 128it+(j%128), 128*(j//128)+p]
    #        j >= 2048: same for w3 (j-2048)
    #   w2b  [EPC, NW2C, 128, W2C*IT*128]
    #        jj = hl*1408 + it*128 + hsub:
    #          w2[slot, 128*(W2C*c+hl)+hsub, 128it+p]
    #   yb   [128, HT*TCAP]   slot s block at col HT*SOFF[s]:
    #        yb[p, HT*SOFF[s] + ht*CAPS[s] + t] = y_slot_s[t, 128ht+p]
    xb = nc.dram_tensor("xb", [128, KT * TCAP], store_dt,
                        kind="ExternalInput").ap()
    w13b = nc.dram_tensor("w13b", [EPC, 128, IT * 2 * KT * 128], store_dt,
                          kind="ExternalInput").ap()
    w2b = nc.dram_tensor("w2b", [EPC, NW2C, 128, W2C * IT * 128], store_dt,
                         kind="ExternalInput").ap()
    yb = nc.dram_tensor("yb", [128, HT * TCAP], store_dt,
                        kind="ExternalOutput").ap()

    with tile.TileContext(nc) as tc:
        with (
            tc.tile_pool(name="xpool", bufs=1) as xpool,
            tc.tile_pool(name="wpool", bufs=3) as wpool,
            tc.tile_pool(name="w2pool", bufs=2) as w2pool,
            tc.tile_pool(name="hhpool", bufs=2 * IT + 1) as hhpool,
            tc.tile_pool(name="evpool", bufs=4) as evpool,
            tc.tile_pool(name="ypool", bufs=2) as ypool,
            tc.tile_pool(name="psAB", bufs=6, space="PSUM") as psAB,
            tc.tile_pool(name="psY", bufs=2, space="PSUM") as psYp,
        ):
            # Resident gathered activations, one tile per slot so matmuls
            # only wait on their own slot's load.
            x_sb = [xpool.tile([128, KT * CAPS[s]], store_dt, tag=f"x{s}",
                               name=f"x_sb{s}")
                    for s in range(EPC)]
            for s in range(EPC):
                nc.scalar.dma_start(
                    x_sb[s][:], xb[:, bass.ds(SOFF[s] * KT, KT * CAPS[s])])

            def body():
              for e in range(EPC):
                cap = CAPS[e]
                hh = []
                for (it0, nits) in W13CH:
                  w13s = wpool.tile([128, nits * 2 * KT * 128], store_dt,
                                    tag="w13s", name="w13s")
                  nc.sync.dma_start(
                      w13s[:], w13b[e][:, bass.ds(it0 * 2 * KT * 128,
                                                  nits * 2 * KT * 128)])
                  for sub in range(nits):
                    off = sub * 2 * KT * 128
                    psA = psAB.tile([128, cap], f32, tag="ps")
                    for k in range(KT):
                        nc.tensor.matmul(
                            psA[:],
                            w13s[:, bass.ds(off + k * 128, 128)],
                            x_sb[e][:, bass.ds(k * cap, cap)],
                            start=(k == 0), stop=(k == KT - 1))
                    psB = psAB.tile([128, cap], f32, tag="ps")
                    for k in range(KT):
                        nc.tensor.matmul(
                            psB[:],
                            w13s[:, bass.ds(off + KT * 128 + k * 128, 128)],
                            x_sb[e][:, bass.ds(k * cap, cap)],
                            start=(k == 0), stop=(k == KT - 1))

                    sA = evpool.tile([128, cap], f32, tag="silu",
                                     name="sA")
                    nc.scalar.activation(
                        sA[:], psA[:], mybir.ActivationFunctionType.Silu)
                    hh_t = hhpool.tile([128, cap], store_dt, tag="hh",
                                       name="hh_t")
                    nc.vector.tensor_mul(hh_t[:], sA[:], psB[:])
                    hh.append(hh_t)

                y_sb = ypool.tile([128, HT * cap], store_dt, tag="y")
                for c in range(NW2C):
                    w2s = w2pool.tile([128, W2C * IT * 128], store_dt,
                                      tag="w2s")
                    nc.sync.dma_start(w2s[:], w2b[e, c])
                    for hl in range(W2C):
                        psy = psYp.tile([128, cap], f32, tag="psy")
                        for it2 in range(IT):
                            nc.tensor.matmul(
                                psy[:],
                                w2s[:, bass.ds(hl * IT * 128 + it2 * 128,
                                               128)],
                                hh[it2][:],
                                start=(it2 == 0), stop=(it2 == IT - 1))
                        ht = c * W2C + hl
                        nc.vector.tensor_copy(
                            y_sb[:, bass.ds(ht * cap, cap)], psy[:])
                nc.gpsimd.dma_start(
                    yb[:, bass.ds(HT * SOFF[e], HT * cap)], y_sb[:])

            if loop_reps > 0:
                with tc.For_i(0, loop_reps, 1,
                              hint_engines=(mybir.EngineType.PE,
                                            mybir.EngineType.SP)):
                    body()
            else:
                for _ in range(repeat):
                    body()
    nc.compile()
    return nc


def get_program(mode=None, repeat=1, loop_reps=0):
    mode = mode or MM_MODE
    key = (mode, repeat, loop_reps)
    if key not in _prog_cache:
        _prog_cache[key] = _build_program(mode, repeat, loop_reps)
    return _prog_cache[key]


_exec_cache = {}


def get_executor(mode=None, repeat=1, loop_reps=0):
    """Build (once) a PJRT executable for the SPMD program. Returns a
    callable: in_maps (list of per-core dicts) -> list of per-core output
    dicts."""
    mode = mode or MM_MODE
    key = (mode, repeat, loop_reps)
    if key in _exec_cache:
        return _exec_cache[key]

    import jax
    from jax.sharding import Mesh, NamedSharding, PartitionSpec
    from jax.experimental.shard_map import shard_map
    from concourse import bass2jax

    bass2jax.install_neuronx_cc_hook()
    nc = get_program(mode, repeat, loop_reps)

    partition_name = (nc.partition_id_tensor.name
                      if nc.partition_id_tensor else None)
    in_names, out_names, out_avals, out_shapes = [], [], [], []
    for alloc in nc.m.functions[0].allocations:
        if not isinstance(alloc, mybir.MemoryLocationSet):
            continue
        name = alloc.memorylocations[0].name
        if alloc.kind == "ExternalInput":
            if name != partition_name:
                in_names.append(name)
        elif alloc.kind == "ExternalOutput":
            shape = tuple(alloc.tensor_shape)
            dtype = mybir.dt.np(alloc.dtype)
            out_names.append(name)
            out_avals.append(jax.core.ShapedArray(shape, dtype))
            out_shapes.append((shape, dtype))
    n_params = len(in_names)
    n_outs = len(out_avals)
    all_in_names = in_names + out_names + (
        [partition_name] if partition_name else [])

    def _body(*args):
        operands = list(args)
        if partition_name is not None:
            operands.append(bass2jax.partition_id_tensor())
        return tuple(bass2jax._bass_exec_p.bind(
            *operands,
            out_avals=tuple(out_avals),
            in_names=tuple(all_in_names),
            out_names=tuple(out_names),
            lowering_input_output_aliases=(),
            sim_require_finite=True,
            sim_require_nnan=True,
            nc=nc,
        ))

    devices = jax.devices()[:N_CORES]
    mesh = Mesh(np.asarray(devices), ("core",))
    sharded = jax.jit(
        shard_map(_body, mesh=mesh,
                  in_specs=(PartitionSpec("core"),) * (n_params + n_outs),
                  out_specs=(PartitionSpec("core"),) * n_outs,
                  check_rep=False),
        donate_argnums=tuple(range(n_params, n_params + n_outs)),
        keep_unused=True)
    shard = NamedSharding(mesh, PartitionSpec("core"))

    def run(in_maps):
        concat_in = [
            np.concatenate([np.asarray(in_maps[c][nm])
                            for c in range(N_CORES)], axis=0)
            for nm in in_names]
        zeros = [np.zeros((N_CORES * s[0], *s[1:]), d)
                 for (s, d) in out_shapes]
        outs = sharded(*[jax.device_put(a, shard) for a in concat_in],
                       *[jax.device_put(z, shard) for z in zeros])
        return [
            {name: np.asarray(outs[i]).reshape(N_CORES, *out_avals[i].shape)[c]
             for i, name in enumerate(out_names)}
            for c in range(N_CORES)]

    run.in_names = in_names
    run.out_names = out_names
    run.out_shapes = out_shapes
    run.sharded = sharded
    run.shard = shard
    _exec_cache[key] = run
    return run


def _route(hidden_states, gate_weight):
    """Replicates the module's router on CPU via jax (bit-compatible with
    the reference implementation)."""
    import jax
    import jax.numpy as jnp
    cpu = jax.devices("cpu")[0]
    with jax.default_device(cpu):
        hs = jnp.asarray(hidden_states)
        gw = jnp.asarray(gate_weight)
        logits = hs @ gw.T
        probs = jax.nn.softmax(logits.astype(jnp.float32), axis=-1)
        group_scores = probs.reshape(T, G, E // G).max(axis=-1)
        _, gidx = jax.lax.top_k(group_scores, TG)
        rows = jnp.arange(T)[:, None]
        gmask = jnp.zeros((T, G), probs.dtype).at[rows, gidx].set(1.0)
        smask = jnp.repeat(gmask, E // G, axis=1)
        tmp_scores = jnp.where(smask > 0, probs, 0.0)
        rw, sel = jax.lax.top_k(tmp_scores, TOPK)
        return np.asarray(sel), np.asarray(rw, dtype=np.float32)


def _np_store_dtype(mode):
    if mode == "bf16":
        import ml_dtypes
        return np.dtype(ml_dtypes.bfloat16)
    if mode == "f16":
        return np.dtype(np.float16)
    return np.dtype(np.float32)


def prep_inputs(hidden_states, w1_weight, w3_weight, w2_weight, sel, mode):
    """Balance experts to (core, slot) by sorted token count, gather
    tokens, and block weights for the device layout.
    Returns (in_maps, assign, overflow) where assign[t,k] = row in the
    global gathered array (core*TCAP + SOFF[slot] + pos) or -1."""
    sdt = _np_store_dtype(mode)
    counts = np.bincount(sel.reshape(-1), minlength=E)
    order = np.argsort(-counts, kind="stable")   # expert ids, desc count
    # expert_of[c][s] = expert id owned by core c, slot s
    expert_of = np.empty((N_CORES, EPC), dtype=np.int64)
    slot_of = np.empty(E, dtype=np.int64)        # expert -> flat slot id
    for s in range(EPC):
        for c in range(N_CORES):
            eid = order[s * N_CORES + c]
            expert_of[c, s] = eid
            slot_of[eid] = c * EPC + s

    assign = np.full((T, TOPK), -1, dtype=np.int64)
    fill = np.zeros(E, dtype=np.int64)
    overflow = []
    tok_of = [np.zeros(CAPS[s % EPC], dtype=np.int64)
              for s in range(N_CORES * EPC)]
    used = [np.zeros(CAPS[s % EPC], dtype=bool)
            for s in range(N_CORES * EPC)]
    for t in range(T):
        for k in range(TOPK):
            e = sel[t, k]
            fs = slot_of[e]
            c, s = fs // EPC, fs % EPC
            p = fill[e]
            if p < CAPS[s]:
                tok_of[fs][p] = t
                used[fs][p] = True
                fill[e] = p + 1
                assign[t, k] = c * TCAP + SOFF[s] + p
            else:
                overflow.append((t, k, e))

    in_maps = []
    for core in range(N_CORES):
        # gathered x per slot -> xb [128, KT*TCAP]
        xbc = np.zeros((128, KT * TCAP), dtype=np.float32)
        for s in range(EPC):
            fs = core * EPC + s
            xg = np.zeros((CAPS[s], H), dtype=np.float32)
            xg[used[fs]] = hidden_states[tok_of[fs][used[fs]]]
            # [t, H] -> [k, p, t] -> [p, k*cap+t]
            blk = (xg.T.reshape(KT, 128, CAPS[s])
                   .transpose(1, 0, 2).reshape(128, KT * CAPS[s]))
            xbc[:, SOFF[s] * KT:SOFF[s] * KT + KT * CAPS[s]] = blk
        xbc = np.ascontiguousarray(xbc).astype(sdt, copy=False)

        es = expert_of[core]                      # expert ids for slots
        # weights: w1/w3 [e, I, H] -> strips [s, it, p(h in kt), kt*128+i]
        w1c = (w1_weight[es].transpose(0, 2, 1)   # [s, H, I]
               .reshape(EPC, KT, 128, IT, 128)    # [s, kt, p, it, i]
               .transpose(0, 3, 2, 1, 4)          # [s, it, p, kt, i]
               .reshape(EPC, IT, 128, KT * 128))
        w3c = (w3_weight[es].transpose(0, 2, 1)
               .reshape(EPC, KT, 128, IT, 128)
               .transpose(0, 3, 2, 1, 4)
               .reshape(EPC, IT, 128, KT * 128))
        w13c = np.ascontiguousarray(
            np.concatenate([w1c, w3c], axis=-1)   # [s, it, p, 4096]
            .transpose(0, 2, 1, 3)                # [s, p, it, 4096]
            .reshape(EPC, 128, IT * 2 * KT * 128)).astype(sdt, copy=False)
        # w2 [e, H, I] -> strips [s, ht, p(i in it), it*128+h] chunked by 4
        w2c = np.ascontiguousarray(
            w2_weight[es].transpose(0, 2, 1)      # [s, I, H]
            .reshape(EPC, IT, 128, HT, 128)       # [s, it, p, ht, h]
            .transpose(0, 3, 2, 1, 4)             # [s, ht, p, it, h]
            .reshape(EPC, NW2C, W2C, 128, IT * 128)   # [s, c, hl, p, ith]
            .transpose(0, 1, 3, 2, 4)             # [s, c, p, hl, ith]
            .reshape(EPC, NW2C, 128, W2C * IT * 128)).astype(sdt, copy=False)
        in_maps.append({"xb": xbc, "w13b": w13c, "w2b": w2c})
    return in_maps, assign, overflow


def combine(results, assign, rw, overflow, hidden_states,
            w1_weight, w3_weight, w2_weight):
    # Global gathered output rows: core-major [N_CORES*TCAP, H]
    ys = []
    for core in range(N_CORES):
        yc = results[core]["yb"].astype(np.float32)   # [128, HT*TCAP]
        ycore = np.empty((TCAP, H), dtype=np.float32)
        for s in range(EPC):
            blk = yc[:, HT * SOFF[s]:HT * SOFF[s] + HT * CAPS[s]]
            # [p, ht*cap+t] -> [t, ht, p] -> [t, H]
            ycore[SOFF[s]:SOFF[s] + CAPS[s]] = (
                blk.reshape(128, HT, CAPS[s]).transpose(2, 1, 0)
                .reshape(CAPS[s], H))
        ys.append(ycore)
    yg = np.concatenate(ys, axis=0)               # [N_CORES*TCAP, H]

    flat = assign.reshape(-1)
    ok = flat >= 0
    picked = np.zeros((T * TOPK, H), dtype=np.float32)
    picked[ok] = yg[flat[ok]]
    out = (picked.reshape(T, TOPK, H)
           * rw[:, :, None]).sum(axis=1).astype(np.float32)

    if overflow:
        for (t, k, e) in overflow:
            x = hidden_states[t]
            h = (x @ w1_weight[e].T)
            h = (h / (1.0 + np.exp(-h))) * (x @ w3_weight[e].T)
            out[t] += rw[t, k] * (h @ w2_weight[e].T)
    return out


def kernel(hidden_states, gate_weight, w1_weight, w3_weight, w2_weight):
    mode = MM_MODE
    runner = get_executor(mode)
    sel, rw = _route(hidden_states, gate_weight)
    in_maps, assign, overflow = prep_inputs(
        hidden_states, w1_weight, w3_weight, w2_weight, sel, mode)
    results = runner(in_maps)
    return combine(results, assign, rw, overflow, hidden_states,
                   w1_weight, w3_weight, w2_weight)
